# revision 8
# baseline (speedup 1.0000x reference)
"""AdditiveAttention (Bahdanau) TRN2 Bass kernel, mask-sparse.

softmax(mask ? tanh(vW + MU) @ v : -inf)  over rows, for
B=32, R=4096, D=1024, data-parallel over batch across 8 NeuronCores.

Sparsity: masked rows contribute exactly-0 probabilities, so the kernel
only computes scores for unmasked rows.  The host compacts each batch's
unmasked rows (index gather, padded to a common multiple of 128), the
device computes the masked softmax over the compacted rows, and the host
scatters the probabilities back (masked positions are 0; an all-masked
row degenerates to the uniform distribution, matching the reference).

Per core (4 batches):
  - load W/U/v once, cast to fp16 (DVE); proj_v = vec @ W via PE (fp16)
    with vec transposed on PE.
  - per (batch, row block): load matrix rows fp32, DVE-cast to fp16,
    PE-transpose 128x128 fp16 tiles into PSUM, DVE-copy to [d, r] fp16
    layout; 8 e-chunk matmul groups (8 fp16 matmuls each) -> PSUM fp32,
    tanh+bias on ScalarE -> fp16 inter, v-dot matmuls -> scores [1, r].
  - per batch: predicated-copy scores over a -100 background (pad mask),
    exp with fused accumulate -> softmax, DMA out fp32.
"""

from contextlib import ExitStack

import numpy as np

import bass_rust
import concourse.bass as bass
import concourse.tile as tile
from concourse import mybir
from concourse import bass_utils

F32 = mybir.dt.float32
F16 = mybir.dt.float16
I8 = mybir.dt.int8

B, R, D = 32, 4096, 1024
NCORES = 8
BPC = B // NCORES          # batches per core
NC_ = D // 128             # d (and e) chunks
NEG = -100.0               # masked logit; exp(-100) underflows to ~0 in fp32

_uid = [0]


def _legalize_waits(nc):
    """This walrus accepts at most 1 sync wait per instruction (2 for
    EventSemaphore); Tile's kernel-tail drain piles all terminal waits onto
    one Drain. Split the excess into wait-only EventSemaphores."""
    for f in nc.m.functions:
        for bb in f.blocks:
            insts = list(bb.instructions)
            new_insts = []
            changed = False
            for inst in insts:
                si = inst.sync_info
                waits = list(si.on_wait) if si is not None else []
                cap = 2 if isinstance(inst, mybir.InstEventSemaphore) else 1
                if len(waits) > cap:
                    changed = True
                    keep, rest = waits[:cap], waits[cap:]
                    for i in range(0, len(rest), 2):
                        _uid[0] += 1
                        ev = mybir.InstEventSemaphore(
                            name=f"lw_{inst.name}_{_uid[0]}", ins=[], outs=[]
                        )
                        ev.engine = inst.engine
                        ev.sync_info = bass_rust.SyncInfo(
                            on_wait=list(rest[i : i + 2]), on_update=[]
                        )
                        new_insts.append(ev)
                    inst.sync_info = bass_rust.SyncInfo(
                        on_wait=keep, on_update=list(si.on_update)
                    )
                new_insts.append(inst)
            if changed:
                bb.instructions = new_insts
    return nc


def _chunks(width, step):
    """[(offset, size), ...] covering [0, width) in steps of `step`."""
    return [(o, min(step, width - o)) for o in range(0, width, step)]


def _emit(nc, Rc):
    blocks = _chunks(Rc, 1024)   # row blocks per batch
    vec_in = nc.dram_tensor("vec", [BPC, D], F32, kind="ExternalInput").ap()
    mat_in = nc.dram_tensor("mat", [BPC, Rc, D], F32, kind="ExternalInput").ap()
    mask_in = nc.dram_tensor("mask", [BPC, Rc], I8, kind="ExternalInput").ap()
    w_in = nc.dram_tensor("w", [D, D], F32, kind="ExternalInput").ap()
    u_in = nc.dram_tensor("u", [D, D], F32, kind="ExternalInput").ap()
    v_in = nc.dram_tensor("v", [D, 1], F32, kind="ExternalInput").ap()
    id_in = nc.dram_tensor("ident", [128, 128], F32, kind="ExternalInput").ap()
    out = nc.dram_tensor("out", [BPC, Rc], F32, kind="ExternalOutput").ap()

    with tile.TileContext(nc) as tc, ExitStack() as ctx:
        consts = ctx.enter_context(tc.tile_pool(name="consts", bufs=1))
        big = ctx.enter_context(tc.tile_pool(name="big", bufs=4))      # 16KB slots
        m16_p = ctx.enter_context(tc.tile_pool(name="m16p", bufs=3))   # 8KB slots
        matT_p = ctx.enter_context(tc.tile_pool(name="matT", bufs=2))  # 16KB slots
        inter_p = ctx.enter_context(tc.tile_pool(name="inter", bufs=3))
        row_p = ctx.enter_context(tc.tile_pool(name="row", bufs=2))
        mask_p = ctx.enter_context(tc.tile_pool(name="maskp", bufs=2))
        tp_ps = ctx.enter_context(tc.tile_pool(name="tp_ps", bufs=2, space="PSUM"))
        pm_ps = ctx.enter_context(tc.tile_pool(name="pm_ps", bufs=2, space="PSUM"))
        sc_ps = ctx.enter_context(tc.tile_pool(name="sc_ps", bufs=1, space="PSUM"))

        # ---- first matrix block's loads win queue priority; the constants
        # (W/U/v/ident) stream in behind them.
        blk0 = blocks[0]
        halves0 = _chunks(blk0[1], 512)
        m16h0 = []
        for h, (h0, hw) in enumerate(halves0):
            m32 = big.tile([128, hw // 128, D], F32, tag="big",
                           name=f"m32_0_0_{h}")
            nc.sync.dma_start(
                m32[:], mat_in[0, h0 : h0 + hw, :].rearrange(
                    "(t p) d -> p t d", p=128))
            m16h0.append(m32)

        u16 = consts.tile([128, NC_, D], F16, tag="u16")
        pv_sb = consts.tile([128, NC_, BPC], F32, tag="pv")
        u_cols = u_in.rearrange("(c p) e -> p c e", p=128)

        def load_ucol(k):
            nc.gpsimd.dma_start(u16[:, :, 128 * k : 128 * (k + 1)],
                                u_cols[:, :, 128 * k : 128 * (k + 1)])

        load_ucol(0)
        w16 = big.tile([128, NC_, D], F16, tag="big", name="w16")
        nc.gpsimd.dma_start(w16[:], w_in.rearrange("(c p) e -> p c e", p=128))
        load_ucol(1)

        ident = consts.tile([128, 128], F32, tag="ident")
        nc.sync.dma_start(ident[:], id_in[:])
        ident16 = consts.tile([128, 128], F16, tag="ident16")
        nc.vector.tensor_copy(ident16[:], ident[:])
        v32 = consts.tile([128, NC_], F32, tag="v32")
        nc.sync.dma_start(v32[:], v_in.rearrange("(c p) one -> p (c one)", p=128))
        v16 = consts.tile([128, NC_], F16, tag="v16")
        nc.vector.tensor_copy(v16[:], v32[:])
        vec_sb = consts.tile([BPC, D], F32, tag="vec")
        nc.sync.dma_start(vec_sb[:], vec_in[:])

        # fp16 casts for block 0 (DVE) — emitted before vecT so the DVE hits
        # them as soon as the data lands
        m16h0c = []
        for h, (h0, hw) in enumerate(halves0):
            m16 = m16_p.tile([128, hw // 128, D], F16, tag="m16",
                             name=f"m16_0_0_{h}")
            nc.vector.tensor_copy(m16[:], m16h0[h][:])
            m16h0c.append(m16)

        vecT16 = consts.tile([128, NC_, BPC], F16, tag="vecT", name="vecT16")

        def emit_vecT():
            for c in range(NC_):
                tp = tp_ps.tile([128, 512], F32, tag="tp", name=f"tpv_{c}")
                nc.tensor.transpose(tp[:, 0:BPC],
                                    vec_sb[:, 128 * c : 128 * (c + 1)],
                                    ident[0:BPC, 0:BPC])
                nc.vector.tensor_copy(vecT16[:, c, :], tp[:, 0:BPC])

        def emit_pv():
            """proj_v — tiny matmuls into one sc-pool tile (the slot is free
            until the first v-dot; sc2 is allocated after, so Tile's WAR
            tracking orders the reuse)."""
            pv = sc_ps.tile([128, NC_, BPC], F32, tag="sc", name="pv")
            for k in range(NC_):
                for c in range(NC_):
                    nc.tensor.matmul(
                        pv[:, k, :],
                        w16[:, c, 128 * k : 128 * (k + 1)],
                        vecT16[:, c, :],
                        start=(c == 0),
                        stop=(c == NC_ - 1),
                    )
                nc.vector.tensor_copy(pv_sb[:, k, :], pv[:, k, :])

        consts_state = {"done": False}

        def emit_wu_consts():
            """Remaining U columns — emitted after the first block's matrix
            loads so those DMAs win queue priority."""
            if consts_state["done"]:
                return
            consts_state["done"] = True
            for k in range(2, NC_):
                load_ucol(k)

        # ---------------- main loop ----------------
        scores_t = {}
        mask_t = {}

        def emit_batch_setup(b):
            if b >= BPC or b in scores_t:
                return
            scores_t[b] = row_p.tile([1, Rc], F32, tag="scores",
                                     name=f"scores_{b}")
            nc.gpsimd.memset(scores_t[b][:], NEG)
            mask_t[b] = mask_p.tile([1, Rc], I8, tag="mask", name=f"mask_{b}")
            nc.sync.dma_start(mask_t[b][:], mask_in[b : b + 1, :])

        emit_batch_setup(0)
        for b in range(BPC):
            scores = scores_t[b]
            mask_sb = mask_t[b]

            for rb, (r0, rblk) in enumerate(blocks):
                sfx = f"{b}_{rb}"
                halves = _chunks(rblk, 512)   # DMA/cast chunks within block
                js = _chunks(rblk, 512)       # PSUM column chunks

                matT = matT_p.tile([128, NC_, rblk], F16, tag="matT",
                                   name=f"matT_{sfx}")
                first = b == 0 and rb == 0
                if first:
                    m16h = m16h0c   # loaded + cast up front
                else:
                    m16h = []
                    for h, (h0, hw) in enumerate(halves):
                        hr = r0 + h0
                        m32 = big.tile([128, hw // 128, D], F32, tag="big",
                                       name=f"m32_{sfx}_{h}")
                        nc.sync.dma_start(
                            m32[:], mat_in[b, hr : hr + hw, :].rearrange(
                                "(t p) d -> p t d", p=128))
                        m16 = m16_p.tile([128, hw // 128, D], F16, tag="m16",
                                         name=f"m16_{sfx}_{h}")
                        nc.vector.tensor_copy(m16[:], m32[:])
                        m16h.append(m16)
                if consts_state["done"] is False and rb == 0:
                    emit_wu_consts()
                if rb == 0:
                    emit_batch_setup(b + 1)

                if first:
                    # split per half so e-chunk matmuls can start on the
                    # first 2MB of matrix data
                    for h, (h0, hw) in enumerate(halves):
                        for c in range(NC_):
                            tp = tp_ps.tile([128, 512], F16, tag="tp",
                                            name=f"tpf_{c}_{h}")
                            for i in range(hw // 128):
                                nc.tensor.transpose(
                                    tp[:, 128 * i : 128 * (i + 1)],
                                    m16h[h][:, i, 128 * c : 128 * (c + 1)],
                                    ident16[:],
                                )
                            nc.vector.tensor_copy(
                                matT[:, c, h0 : h0 + hw], tp[:, 0:hw])
                    emit_vecT()
                else:
                    for c in range(NC_):
                        tp = tp_ps.tile([128, rblk], F16, tag="tp",
                                        name=f"tp_{sfx}_{c}")
                        for h, (h0, hw) in enumerate(halves):
                            for i in range(hw // 128):
                                nc.tensor.transpose(
                                    tp[:, h0 + 128 * i : h0 + 128 * (i + 1)],
                                    m16h[h][:, i, 128 * c : 128 * (c + 1)],
                                    ident16[:],
                                )
                        nc.vector.tensor_copy(matT[:, c, :], tp[:])

                # per e-chunk: proj_m -> tanh -> v-dot
                # (vdot(k) emitted after pm(k+1) so the PE never waits on
                # the tanh that feeds it)
                state = {"sc2": None}
                inters = []

                def get_sch():
                    if state["sc2"] is None:
                        state["sc2"] = sc_ps.tile([1, rblk], F32, tag="sc",
                                                  name=f"sc_{sfx}")
                    return [state["sc2"][:, j0 : j0 + jw] for j0, jw in js]

                def emit_vdot(k):
                    sch = get_sch()
                    for j, (j0, jw) in enumerate(js):
                        nc.tensor.matmul(
                            sch[j][:],
                            v16[:, k : k + 1],
                            inters[k][:, j0 : j0 + jw],
                            start=(k == 0),
                            stop=(k == NC_ - 1),
                        )

                for k in range(NC_):
                    pm = pm_ps.tile([128, rblk], F32, tag="pm",
                                    name=f"pm_{sfx}_{k}")
                    if first:
                        # j-outer: the j=0 matmuls only need the first
                        # half-block of matT
                        for j0, jw in js:
                            for c in range(NC_):
                                nc.tensor.matmul(
                                    pm[:, j0 : j0 + jw],
                                    u16[:, c, 128 * k : 128 * (k + 1)],
                                    matT[:, c, j0 : j0 + jw],
                                    start=(c == 0),
                                    stop=(c == NC_ - 1),
                                )
                        if k == 0:
                            emit_pv()
                    else:
                        for c in range(NC_):
                            for j0, jw in js:
                                nc.tensor.matmul(
                                    pm[:, j0 : j0 + jw],
                                    u16[:, c, 128 * k : 128 * (k + 1)],
                                    matT[:, c, j0 : j0 + jw],
                                    start=(c == 0),
                                    stop=(c == NC_ - 1),
                                )
                    if k >= 1:
                        emit_vdot(k - 1)
                    inter = inter_p.tile([128, rblk], F16, tag="inter",
                                         name=f"inter_{sfx}_{k}")
                    nc.scalar.activation(
                        inter[:], pm[:], mybir.ActivationFunctionType.Tanh,
                        bias=pv_sb[:, k, b : b + 1], scale=1.0,
                    )
                    inters.append(inter)
                emit_vdot(NC_ - 1)
                # masked copy into scores row (background is NEG)
                sch = get_sch()
                for j, (j0, jw) in enumerate(js):
                    nc.vector.copy_predicated(
                        scores[:, r0 + j0 : r0 + j0 + jw],
                        mask_sb[:, r0 + j0 : r0 + j0 + jw],
                        sch[j][:],
                    )

            # softmax over the row
            ex = row_p.tile([1, Rc], F32, tag="ex", name=f"ex_{b}")
            ssum = consts.tile([1, 1], F32, tag="ssum", name=f"ssum_{b}")
            nc.scalar.activation(
                ex[:], scores[:], mybir.ActivationFunctionType.Exp,
                bias=0.0, scale=1.0, accum_out=ssum[:],
            )
            rec = consts.tile([1, 1], F32, tag="rec", name=f"rec_{b}")
            nc.vector.reciprocal(rec[:], ssum[:])
            if b == BPC - 1:
                # tail-exposed: split the scale across DVE and ACT
                nc.vector.tensor_scalar_mul(ex[:, 0 : Rc // 2],
                                            ex[:, 0 : Rc // 2], rec[:])
                nc.scalar.mul(ex[:, Rc // 2 : Rc], ex[:, Rc // 2 : Rc], rec[:])
            else:
                nc.gpsimd.tensor_scalar_mul(ex[:], ex[:], rec[:])
            nc.sync.dma_start(out[b : b + 1, :], ex[:])

    return nc


_NC_CACHE = {}


def _get_nc(Rc):
    if Rc not in _NC_CACHE:
        nc = bass.Bass("TRN2", target_bir_lowering=False, debug=False)
        _emit(nc, Rc)
        _legalize_waits(nc)
        _NC_CACHE[Rc] = nc
    return _NC_CACHE[Rc]


def make_plan(matrix_mask):
    """Per-batch unmasked row indices + common padded row count Rc."""
    m = np.asarray(matrix_mask) != 0
    idxs = [np.nonzero(m[b])[0] for b in range(m.shape[0])]
    mx = max(len(i) for i in idxs)
    Rc = min(R, max(128, -(-mx // 128) * 128))
    return idxs, Rc


def make_in_maps(vector, matrix, matrix_mask, w_matrix, u_matrix, v_vector,
                 idxs, Rc):
    ident = np.eye(128, dtype=np.float32)
    vector = np.ascontiguousarray(vector, dtype=np.float32)
    matrix = np.asarray(matrix)
    w = np.ascontiguousarray(w_matrix, dtype=np.float32)
    u = np.ascontiguousarray(u_matrix, dtype=np.float32)
    v = np.ascontiguousarray(v_vector, dtype=np.float32)
    in_maps = []
    for c in range(NCORES):
        mat_c = np.empty((BPC, Rc, D), dtype=np.float32)
        mask_c = np.zeros((BPC, Rc), dtype=np.int8)
        for j in range(BPC):
            gb = c * BPC + j
            idx = idxs[gb]
            n = len(idx)
            pad = np.zeros(Rc - n, dtype=np.intp) if n == 0 else \
                np.full(Rc - n, idx[0], dtype=np.intp)
            idx_pad = np.concatenate([idx.astype(np.intp), pad])
            mat_c[j] = matrix[gb][idx_pad]
            mask_c[j, :n] = 1
        in_maps.append({
            "vec": vector[c * BPC : (c + 1) * BPC],
            "mat": mat_c,
            "mask": mask_c,
            "w": w,
            "u": u,
            "v": v,
            "ident": ident,
        })
    return in_maps


def scatter_out(results, idxs, Rc):
    out = np.zeros((B, R), dtype=np.float32)
    for c in range(NCORES):
        dev = results[c]["out"]
        for j in range(BPC):
            gb = c * BPC + j
            idx = idxs[gb]
            if len(idx) == 0:
                out[gb, :] = 1.0 / R   # softmax of all-equal (-1e9) logits
            else:
                out[gb, idx] = dev[j, : len(idx)]
    return out


def kernel(vector, matrix, matrix_mask, w_matrix, u_matrix, v_vector):
    idxs, Rc = make_plan(matrix_mask)
    nc = _get_nc(Rc)
    in_maps = make_in_maps(vector, matrix, matrix_mask, w_matrix, u_matrix,
                           v_vector, idxs, Rc)
    res = bass_utils.run_bass_kernel_spmd(nc, in_maps, core_ids=list(range(NCORES)))
    return scatter_out(res.results, idxs, Rc)
